# revision 1
# baseline (speedup 1.0000x reference)
"""TRN2 Bass kernel for nn_DAAAgregationLayer (GNN message passing).

out = (segment_sum((F[i]+F[j]) * (cos * dist[n,i] * dist[n,j]), pair_node)) @ W + b

Sharding (8 NeuronCores): pair_node is sorted, so pairs and their destination
nodes are split into 8 contiguous node ranges (1250 nodes/core); the segment
sum is fully local to each core and the small GEMM runs on each core's output
slice (no collectives). dist is row-sharded by destination node; features, W,
b are replicated. On device, per core:
  - pair feature rows F[i], F[j] are fetched from HBM with the gpsimd
    dma_gather custom op (512B descriptors, int16 row ids),
  - dist values are fetched as 64-element chunks from 8 column strips of the
    dist row-shard (dma_gather, chunk index = nhat*20 + col//64), and the
    target column is selected on the vector engine via one-hot compare +
    multiply + free-axis reduce,
  - w = cos*d_i*d_j scales a one-hot window matrix WS (column = destination
    node within a 128-node window); PE matmuls WS^T @ F_gathered compute
    gather + weight + segment-sum in one pass, accumulated per node block,
  - a PE transpose + matmul applies W; the vector engine adds b.
All FLOPs and all data-dependent gathers run on the NeuronCores; the host only
computes index/layout metadata, shards inputs, and concatenates the 8 output
slices.
"""
import sys
for _p in ('/opt/trn_rl_repo', '/root/.axon_site/_ro/trn_rl_repo'):
    if _p not in sys.path:
        sys.path.append(_p)
import numpy as np

import concourse.bass as bass
import concourse.tile as tile
from concourse import bacc, mybir
from concourse.masks import make_identity

P = 128
DIM = 128
N_NODES = 10000
N_CORES = 8
NW = N_NODES // N_CORES          # 1250
SB = 1280                        # strip width in cols
NSTRIP = 8
CPS = SB // 128                  # 10 chunks of 128 (512B) per strip row
NV = 10                          # windows == psum node blocks (128 wide)
NBLK = (NW + P - 1) // P         # 10 psum node blocks
NCELL = NSTRIP * NSTRIP * NV     # (si,sj,v) cells = 1280


def cell_of(si, sj, v):
    return (si * NSTRIP + sj) * NV + v


def pack_core(nhat, i64, j64, cos, quota):
    """Pack one core's pairs into the uniform (si,sj,v) chunk grid.
    quota: [NCELL] chunks per cell (uniform across cores) or None->counts."""
    si = i64 // SB
    sj = j64 // SB
    v = nhat // P
    cell = (si * NSTRIP + sj) * NV + v
    order = np.argsort(cell, kind="stable")
    nhat, i64, j64, cos, si, sj, cell = (a[order] for a in (nhat, i64, j64, cos, si, sj, cell))
    cnt = np.bincount(cell, minlength=NCELL)
    if quota is None:
        return (cnt + P - 1) // P

    C = int(quota.sum())
    Npad = C * P
    g_fi = np.zeros(Npad, dtype=np.int16)
    g_fj = np.zeros(Npad, dtype=np.int16)
    g_di = np.zeros(Npad, dtype=np.int16)
    g_dj = np.zeros(Npad, dtype=np.int16)
    cix_i = np.zeros((P, C), dtype=np.float32)
    cix_j = np.zeros((P, C), dtype=np.float32)
    cosv = np.zeros((P, C), dtype=np.float32)
    nrel = np.full((P, C), -1000.0, dtype=np.float32)

    cstart = np.concatenate([[0], np.cumsum(cnt)[:-1]])
    ci = 0
    for cl in range(NCELL):
        s = int(cstart[cl]); n = int(cnt[cl])
        for q in range(int(quota[cl])):
            lo = s + q * P
            L = max(0, min(P, n - q * P))
            if L:
                idx = np.arange(lo, lo + L)
                g = ci * P + np.arange(L)
                g_fi[g] = i64[idx]
                g_fj[g] = j64[idx]
                irel = i64[idx] - si[idx] * SB
                jrel = j64[idx] - sj[idx] * SB
                g_di[g] = nhat[idx] * CPS + irel // 128
                g_dj[g] = nhat[idx] * CPS + jrel // 128
                cix_i[:L, ci] = (irel % 128).astype(np.float32)
                cix_j[:L, ci] = (jrel % 128).astype(np.float32)
                cosv[:L, ci] = cos[idx]
                wb = (cl % NV) * P
                nrel[:L, ci] = (nhat[idx] - wb).astype(np.float32)
            ci += 1
    assert ci == C
    return dict(fi=g_fi, fj=g_fj, di=g_di, dj=g_dj,
                cix_i=cix_i, cix_j=cix_j, cosv=cosv, nrel=nrel)


def wrap_idx(lst):
    n = len(lst)
    assert n % 16 == 0
    return np.ascontiguousarray(np.tile(lst.reshape(n // 16, 16).T, (8, 1)))


def make_inputs(features, dist, pair_node, pair_i, pair_j, pair_cos, W, b):
    """Returns (in_maps list of dicts, quota) for the 8 cores."""
    bounds = np.searchsorted(pair_node, np.arange(N_CORES + 1) * NW)
    raw = []
    for c in range(N_CORES):
        s, e = int(bounds[c]), int(bounds[c + 1])
        raw.append((pair_node[s:e].astype(np.int64) - c * NW,
                    pair_i[s:e].astype(np.int64), pair_j[s:e].astype(np.int64),
                    pair_cos[s:e].astype(np.float32)))
    quota = np.max([pack_core(*r, None) for r in raw], axis=0)
    quota = np.maximum(quota, (np.arange(NCELL) % NV == np.arange(NCELL) % NV) * 0)
    # ensure every window v has at least one chunk overall
    for v in range(NV):
        if quota[np.arange(NCELL) % NV == v].sum() == 0:
            quota[v] = 1
    iota64 = np.tile(np.arange(P, dtype=np.float32), (P, 1))
    brep = np.tile(b.reshape(1, DIM).astype(np.float32), (P, 1))
    dist_pad = np.zeros((NW, NSTRIP * SB), dtype=np.float32)
    in_maps = []
    for c in range(N_CORES):
        pk = pack_core(*raw[c], quota)
        dp = dist_pad.copy()
        dp[:, :N_NODES] = dist[c * NW:(c + 1) * NW]
        m = {
            "features": np.ascontiguousarray(features.astype(np.float32)),
            "Wm": np.ascontiguousarray(W.astype(np.float32)),
            "brep": brep,
            "iota64": iota64,
            "fidx_i": wrap_idx(pk["fi"]), "fidx_j": wrap_idx(pk["fj"]),
            "didx_i": wrap_idx(pk["di"]), "didx_j": wrap_idx(pk["dj"]),
            "cix_i": pk["cix_i"], "cix_j": pk["cix_j"],
            "cosv": pk["cosv"], "nrel": pk["nrel"],
        }
        for s in range(NSTRIP):
            m[f"strip{s}"] = np.ascontiguousarray(
                dp[:, s * SB:(s + 1) * SB].reshape(NW * CPS, 128))
        in_maps.append(m)
    return in_maps, quota


def build_program(quota, n_cores=N_CORES):
    C = int(quota.sum())
    IW = (C * P) // 16           # idx tile cols
    nc = bacc.Bacc("TRN2", target_bir_lowering=False, debug=False, num_devices=n_cores)
    dt = mybir.dt
    feat = nc.dram_tensor("features", [N_NODES, DIM], dt.float32, kind="ExternalInput").ap()
    Wm = nc.dram_tensor("Wm", [DIM, DIM], dt.float32, kind="ExternalInput").ap()
    brep = nc.dram_tensor("brep", [P, DIM], dt.float32, kind="ExternalInput").ap()
    iota64 = nc.dram_tensor("iota64", [P, P], dt.float32, kind="ExternalInput").ap()
    strips = [nc.dram_tensor(f"strip{s}", [NW * CPS, 128], dt.float32, kind="ExternalInput").ap()
              for s in range(NSTRIP)]
    dins = {}
    for nm in ["fidx_i", "fidx_j", "didx_i", "didx_j"]:
        dins[nm] = nc.dram_tensor(nm, [P, IW], dt.int16, kind="ExternalInput").ap()
    for nm in ["cix_i", "cix_j", "cosv", "nrel"]:
        dins[nm] = nc.dram_tensor(nm, [P, C], dt.float32, kind="ExternalInput").ap()
    out = nc.dram_tensor("out", [NW, DIM], dt.float32, kind="ExternalOutput").ap()

    # per-cell chunk col offsets
    cell_off = np.concatenate([[0], np.cumsum(quota)[:-1]]).astype(int)

    with tile.TileContext(nc) as tc:
        import contextlib
        with contextlib.ExitStack() as ctx:
            const = ctx.enter_context(tc.tile_pool(name="const", bufs=1))
            big = ctx.enter_context(tc.tile_pool(name="big", bufs=1))
            gat = ctx.enter_context(tc.tile_pool(name="gat", bufs=2))
            sel = ctx.enter_context(tc.tile_pool(name="sel", bufs=1))
            eph = ctx.enter_context(tc.tile_pool(name="eph", bufs=2))
            psp = ctx.enter_context(tc.tile_pool(name="psum", bufs=4, space="PSUM"))
            ps2 = ctx.enter_context(tc.tile_pool(name="psum2", bufs=2, space="PSUM"))

            # ---- constants / streams ----
            iota_t = const.tile([P, P], dt.float32)
            nc.sync.dma_start(iota_t[:], iota64[:])
            W_t = const.tile([P, DIM], dt.float32)
            nc.sync.dma_start(W_t[:], Wm[:])
            b_t = const.tile([P, DIM], dt.float32)
            nc.sync.dma_start(b_t[:], brep[:])
            ident = const.tile([P, P], dt.float32)
            make_identity(nc, ident[:])
            str_t = {}
            for nm in ["cix_i", "cix_j", "cosv", "nrel"]:
                t_ = big.tile([P, C], dt.float32, tag=nm)
                nc.sync.dma_start(t_[:], dins[nm][:])
                str_t[nm] = t_

            aggs = big.tile([P, NBLK * DIM], dt.float32, tag="aggs")
            nc.vector.memset(aggs[:], 0.0)

            # ---- main loop over (si,sj) buckets ----
            for sij in range(NSTRIP * NSTRIP):
                si, sj = sij // NSTRIP, sij % NSTRIP
                c0 = int(cell_off[sij * NV])
                Cb = int(quota[sij * NV:(sij + 1) * NV].sum())
                if Cb == 0:
                    continue
                ni = Cb * P
                # stream idx slices for this bucket (wrapped space: 8 cols/chunk)
                it = {}
                for nm in ["fidx_i", "fidx_j", "didx_i", "didx_j"]:
                    t_ = gat.tile([P, Cb * 8], dt.int16, tag=f"it_{nm}")
                    nc.sync.dma_start(t_[:], dins[nm][:, c0 * 8:(c0 + Cb) * 8])
                    it[nm] = t_
                fgi = gat.tile([P, Cb, DIM], dt.float32, tag="fgi")
                fgj = gat.tile([P, Cb, DIM], dt.float32, tag="fgj")
                dgi = gat.tile([P, Cb, 128], dt.float32, tag="dgi")
                dgj = gat.tile([P, Cb, 128], dt.float32, tag="dgj")
                for (dst, src_ap, inm) in ((fgi, feat, "fidx_i"), (fgj, feat, "fidx_j"),
                                           (dgi, strips[si], "didx_i"),
                                           (dgj, strips[sj], "didx_j")):
                    for k0 in range(0, Cb, 4):
                        kn = min(4, Cb - k0)
                        nik = kn * P
                        nc.gpsimd.dma_gather(
                            out_ap=dst[:, k0:k0 + kn, :], in_ap=src_ap,
                            idxs_ap=it[inm][:, k0 * 8:(k0 + kn) * 8],
                            num_idxs=nik, num_idxs_reg=nik,
                            elem_size=dst.shape[2])

                # select dist values: dval = sum_k (iota==cix) * dg
                dval = {}
                for side, dg in (("i", dgi), ("j", dgj)):
                    cix = str_t[f"cix_{side}"]
                    eq = sel.tile([P, Cb, 128], dt.float32, tag="eq")
                    for c_ in range(Cb):
                        nc.vector.tensor_scalar(
                            out=eq[:, c_, :], in0=iota_t[:],
                            scalar1=cix[:, c0 + c_:c0 + c_ + 1],
                            scalar2=None,
                            op0=mybir.AluOpType.is_equal)
                    nc.vector.tensor_tensor(out=eq[:], in0=eq[:], in1=dg[:],
                                            op=mybir.AluOpType.mult)
                    dv = sel.tile([P, Cb], dt.float32, tag=f"dv{side}")
                    nc.vector.tensor_reduce(out=dv[:].rearrange("p (c o) -> p c o", o=1),
                                            in_=eq[:], axis=mybir.AxisListType.X,
                                            op=mybir.AluOpType.add)
                    dval[side] = dv

                # w = cos * dvi * dvj
                wv = sel.tile([P, Cb], dt.float32, tag="wv")
                nc.vector.tensor_tensor(out=wv[:], in0=dval["i"][:], in1=dval["j"][:],
                                        op=mybir.AluOpType.mult)
                nc.vector.tensor_tensor(out=wv[:], in0=wv[:],
                                        in1=str_t["cosv"][:, c0:c0 + Cb],
                                        op=mybir.AluOpType.mult)
                # WS one-hot: ws[p, c, k] = (nrel[p,c]==k) * w[p,c]
                ws = sel.tile([P, Cb, P], dt.float32, tag="ws")
                for c_ in range(Cb):
                    nc.vector.tensor_scalar(
                        out=ws[:, c_, :], in0=iota_t[:],
                        scalar1=str_t["nrel"][:, c0 + c_:c0 + c_ + 1],
                        op0=mybir.AluOpType.is_equal,
                        scalar2=wv[:, c_:c_ + 1],
                        op1=mybir.AluOpType.mult)

                # matmuls: per chunk, window v; accumulate via DVE into aggs
                ci = 0
                for v_ in range(NV):
                    nq = int(quota[sij * NV + v_])
                    for q in range(nq):
                        cc = ci + q
                        scr = psp.tile([P, DIM], dt.float32, tag="scr")
                        nc.tensor.matmul(out=scr[:], lhsT=ws[:, cc, :], rhs=fgi[:, cc, :],
                                         start=True, stop=False)
                        nc.tensor.matmul(out=scr[:], lhsT=ws[:, cc, :], rhs=fgj[:, cc, :],
                                         start=False, stop=True)
                        a_sl = aggs[:, v_ * DIM:(v_ + 1) * DIM]
                        nc.vector.tensor_tensor(out=a_sl, in0=a_sl, in1=scr[:],
                                                op=mybir.AluOpType.add)
                    ci += nq

            # ---- epilogue: out = agg @ W + b per block ----
            for b_ in range(NBLK):
                rows = min(P, NW - b_ * P)
                agg = eph.tile([P, DIM], dt.float32, tag="agg")
                nc.vector.tensor_copy(agg[:rows], aggs[:rows, b_ * DIM:(b_ + 1) * DIM])
                aggT_p = ps2.tile([P, P], dt.float32, tag="aggTp")
                nc.tensor.transpose(out=aggT_p[:, :rows], in_=agg[:rows, :], identity=ident[:rows, :rows])
                aggT = eph.tile([P, P], dt.float32, tag="aggT")
                nc.vector.tensor_copy(aggT[:, :rows], aggT_p[:, :rows])
                o2p = ps2.tile([P, DIM], dt.float32, tag="o2p")
                nc.tensor.matmul(out=o2p[:rows, :], lhsT=aggT[:, :rows], rhs=W_t[:],
                                 start=True, stop=True)
                o2 = eph.tile([P, DIM], dt.float32, tag="o2")
                nc.vector.tensor_tensor(out=o2[:rows], in0=o2p[:rows], in1=b_t[:rows],
                                        op=mybir.AluOpType.add)
                nc.sync.dma_start(out[b_ * P:b_ * P + rows], o2[:rows])
    nc.compile()
    return nc


def kernel(features, dist, pair_node, pair_i, pair_j, pair_cos, W, b):
    from concourse.bass_utils import run_bass_kernel_spmd
    features = np.asarray(features); dist = np.asarray(dist)
    pair_node = np.asarray(pair_node); pair_i = np.asarray(pair_i)
    pair_j = np.asarray(pair_j); pair_cos = np.asarray(pair_cos)
    W = np.asarray(W); b = np.asarray(b)
    in_maps, quota = make_inputs(features, dist, pair_node, pair_i, pair_j,
                                 pair_cos, W, b)
    nc = build_program(quota, n_cores=N_CORES)
    res = run_bass_kernel_spmd(nc, in_maps, list(range(N_CORES)))
    out = np.concatenate([res.results[c]["out"] for c in range(N_CORES)], axis=0)
    return np.ascontiguousarray(out.astype(np.float32))

